# revision 41
# baseline (speedup 1.0000x reference)
"""Trainium2 Bass kernel for single-head attention + output projection + residual.

Math per batch element b (N=2048, D=512, U=128):
    Q = x @ W_q; K = x @ W_k; V = x @ W_v
    S = Q @ K.T / sqrt(U); A = softmax(S, axis=-1)
    out = (A @ V) @ W_o + b_o + x

Distribution: data-parallel over batch — 8 batch elements, one per NeuronCore.

v4 design:
- Deep software pipeline: all scores of a query-half issued back-to-back,
  ScalarE exp chases through a PSUM pool into fp8 e-tiles (paired [128,2,HQ]).
- Projections in fp8 DoubleRow: x and W_q/k/v quantized host-side (x*32,
  W*256); two d-chunks contracted per instruction. Also halves the
  startup-critical xT DMA (1MB instead of 2MB).
- ctx accumulation in fp8 DoubleRow: two key-blocks per instruction
  (V pairs [128,2,U] x e pairs). The x/W scale is absorbed by a V-copy
  rescale (1/256) and W_o*(1/32) host-side.
- Softmax denominator on the tensor engine: DoubleRow ones-matmuls against
  the same fp8 e-pairs accumulate den in PSUM (no DVE Esum chain); r=1/den
  recovered via PE transposes of the den rows.
- exp folds the combined scale 1/(sqrt(U)*8192^2) and a -1 bias shift
  (cancels in normalization); bias+residual folded host-side (xb = x+b_o).

Softmax max-subtraction is skipped: scores/sqrt(U) are bounded (~±6) for any
well-scaled input; exp output with -1 shift stays under fp8e4's ±240 range.
"""

import numpy as np
import ml_dtypes

import concourse.bass as bass
import concourse.tile as tile
from concourse import bacc, mybir
from concourse.bass_utils import run_bass_kernel_spmd

N = 2048
D = 512
U = 128
NB = N // 128  # 16 query/key blocks
DC = D // 128  # 4 d-chunks
NS = N // 512  # 4 free-dim slices of 512
HQ = N // 2  # queries per half
NP = NB // 2  # 8 key-block pairs
X_SCALE = 32.0
W_SCALE = 256.0
QK_SCALE = X_SCALE * W_SCALE  # each of Q,K carries this factor
INV_SQRT_U = 1.0 / np.sqrt(U)
EXP_SHIFT = -2.5  # exp(s-2.5): cancels in normalization; keeps max logits
# (~5.7 + fp8 quant noise) safely under fp8e4's 240 ceiling (overflow = NaN)

F32 = mybir.dt.float32
BF16 = mybir.dt.bfloat16
FP8 = mybir.dt.float8e4

DR = mybir.MatmulPerfMode.DoubleRow


def build_attention_nc():
    nc = bacc.Bacc("TRN2", target_bir_lowering=False, debug=False)

    xb_ext = nc.declare_dram_parameter("xb", [N, D], BF16, isOutput=False)
    # host layout [p, ns, c, n5] fp8 (x*32): 2KB/partition runs per ns-piece
    xT_ext = nc.declare_dram_parameter("xT", [128, NS * DC * 512], FP8, isOutput=False)
    # w layouts [p, c, u] fp8 (w*256); DR pairs c-chunks
    wq_ext = nc.declare_dram_parameter("wq", [128, D], FP8, isOutput=False)
    wk_ext = nc.declare_dram_parameter("wk", [128, D], FP8, isOutput=False)
    wv_ext = nc.declare_dram_parameter("wv", [128, D], FP8, isOutput=False)
    wo_ext = nc.declare_dram_parameter("wo", [U, D], BF16, isOutput=False)
    out_ext = nc.declare_dram_parameter("out", [N, D], BF16, isOutput=True)

    with tile.TileContext(nc) as tc:
        _build_body(nc, tc, xb_ext, xT_ext, wq_ext, wk_ext, wv_ext, wo_ext, out_ext)
    nc.compile()
    return nc


def _build_body(nc, tc, xb_ext, xT_ext, wq_ext, wk_ext, wv_ext, wo_ext, out_ext):
    from contextlib import ExitStack

    with ExitStack() as ctx:
        const = ctx.enter_context(tc.tile_pool(name="const", bufs=1))

        # ---- loads: weights first (they gate the first matmuls) ----
        wq_sb = const.tile([128, DC // 2, 2, U], FP8)  # [d-in-chunk, g, t, u]
        wk_sb = const.tile([128, DC // 2, 2, U], FP8)
        wv_sb = const.tile([128, DC // 2, 2, U], FP8)
        wo_sb = const.tile([U, D], BF16)
        nc.scalar.dma_start(wk_sb[:], wk_ext.ap())
        nc.scalar.dma_start(wq_sb[:], wq_ext.ap())
        nc.gpsimd.dma_start(wv_sb[:], wv_ext.ap())
        nc.gpsimd.dma_start(wo_sb[:], wo_ext.ap())

        ones8_sb = const.tile([128, 2, 32], FP8)
        nc.vector.memset(ones8_sb[:], 1.0)
        ident_sb = const.tile([1, 1], F32)
        nc.vector.memset(ident_sb[:], 1.0)
        eshift_sb = const.tile([128, 1], F32)
        nc.vector.memset(eshift_sb[:], EXP_SHIFT)
        # force the exp activation table load while DMAs are in flight
        scratch = const.tile([128, 1], F32)
        nc.scalar.activation(
            scratch[:], eshift_sb[:], mybir.ActivationFunctionType.Exp, scale=1.0
        )

        # xT fp8 in SBUF as [p, ns, c, n5]; 4 ns-pieces, 2KB/partition runs
        xT_sb = const.tile([128, NS, DC, 512], FP8)
        xT_r = xT_ext.ap().rearrange("p (ns c n) -> p ns c n", ns=NS, c=DC)
        for ns in range(NS):
            nc.sync.dma_start(xT_sb[:, ns], xT_r[:, ns])

        # xb = x + b_o precomputed on host; needed only for the epilogue.
        xb_sb = const.tile([128, NB, D], BF16)
        xb_r = xb_ext.ap().rearrange("(nb p) d -> p nb d", p=128)
        nc.sync.dma_start(xb_sb[0:64], xb_r[0:64])
        nc.gpsimd.dma_start(xb_sb[64:128], xb_r[64:128])

        QT_sb = const.tile([U, N], BF16)
        KT_sb = const.tile([U, N], BF16)
        V_sb = const.tile([128, NB, U], FP8)  # kb-pair p at [:, 2p:2p+2, :]
        ctxT_sb = const.tile([U, N], BF16)
        den_sb = [const.tile([1, HQ], F32, name=f"den_sb_{h}") for h in range(2)]
        r_sb = const.tile([128, NB], F32)

        # den accumulators: DR matmul dst must sit at partition base 0, so
        # each (h, j) accumulator gets its own bank ([32, 512] tile, base 0).
        # bufs=2 rotates: h1's tiles reuse h0's banks after den_finalize(0).
        den_pool = ctx.enter_context(
            tc.tile_pool(name="den_ps", bufs=2, space="PSUM")
        )
        den_ps = {}

        def proj_slice(pool, w_sb, oT, ns):
            # fp8 inputs, normal mode (DR ldweights overhead loses here)
            ps = pool.tile([128, 512], F32, tag="ps", name=f"pp_{oT.tensor.name}_{ns}")
            for c in range(DC):
                nc.tensor.matmul(
                    ps[:],
                    lhsT=w_sb[:, c // 2, c % 2, :],
                    rhs=xT_sb[:, ns, c, :],
                    start=(c == 0),
                    stop=(c == DC - 1),
                )
            nc.vector.tensor_copy(oT[:, ns * 512:(ns + 1) * 512], ps[:])

        def make_v(pool, g4):
            # 4 key-blocks' V projections into one PSUM tile, one rescaled
            # fp8 copy out (V carries x*32 * w*256; keep V*32 -> mul 1/256)
            ps = pool.tile([128, 512], F32, tag="ps", name=f"v_{g4}")
            for q in range(4):
                kb = g4 * 4 + q
                ns, n0 = divmod(kb * 128, 512)
                for c in range(DC):
                    nc.tensor.matmul(
                        ps[:, q * 128:(q + 1) * 128],
                        lhsT=xT_sb[:, ns, c, n0:n0 + 128],
                        rhs=wv_sb[:, c // 2, c % 2, :],
                        start=(c == 0),
                        stop=(c == DC - 1),
                        skip_group_check=True,
                    )
            nc.vector.tensor_scalar(
                V_sb[:, g4 * 4:(g4 + 1) * 4, :],
                ps[:],
                1.0 / W_SCALE,
                None,
                op0=mybir.AluOpType.mult,
            )

        e_t = [None] * NB  # paired e-tiles [128, 2, HQ], 8 per half

        def den_mms(h, pr):
            for j in range(2):
                if pr == 0:
                    den_ps[(h, j)] = den_pool.tile(
                        [32, 512], F32, tag="den", name=f"den_{h}_{j}"
                    )
                nc.tensor.matmul(
                    den_ps[(h, j)][:],
                    lhsT=ones8_sb[:],
                    rhs=e_t[h * NP + pr][:, :, j * 512:(j + 1) * 512],
                    start=(pr == 0),
                    stop=(pr == NP - 1),
                    perf_mode=DR,
                    skip_group_check=True,
                )

        # ---- phase 1: projections + all scores/exp + den-h0 ----
        with (
            tc.tile_pool(name="proj_ps", bufs=2, space="PSUM") as pp,
            tc.tile_pool(name="s_ps", bufs=2, space="PSUM") as sp,
            tc.tile_pool(name="e_sb", bufs=16) as ep,
        ):
            proj_slice(pp, wk_sb, KT_sb, 0)
            proj_slice(pp, wq_sb, QT_sb, 0)
            proj_slice(pp, wq_sb, QT_sb, 1)

            def scores_block(h, kb):
                pr = (h * NB + kb) // 2
                t = kb % 2
                q0 = h * HQ
                if t == 0:
                    e_t[pr] = ep.tile([128, 2, HQ], FP8, tag="e", name=f"e_{h}_{kb}")
                s_ps = sp.tile([128, HQ], F32, tag="s", name=f"s_{h}_{kb}")
                for j in range(2):
                    nc.tensor.matmul(
                        s_ps[:, j * 512:(j + 1) * 512],
                        lhsT=KT_sb[:, kb * 128:(kb + 1) * 128],
                        rhs=QT_sb[:, q0 + j * 512:q0 + (j + 1) * 512],
                        start=True,
                        stop=True,
                    )
                nc.scalar.activation(
                    e_t[pr][:, t, :],
                    s_ps[:],
                    mybir.ActivationFunctionType.Exp,
                    bias=eshift_sb[:],
                    scale=INV_SQRT_U / (QK_SCALE * QK_SCALE),
                )

            for kb in range(4):
                scores_block(0, kb)
            proj_slice(pp, wk_sb, KT_sb, 1)
            for kb in range(4, 8):
                scores_block(0, kb)
            proj_slice(pp, wk_sb, KT_sb, 2)
            for kb in range(8, 12):
                scores_block(0, kb)
            proj_slice(pp, wk_sb, KT_sb, 3)
            for kb in range(12, 16):
                scores_block(0, kb)

            proj_slice(pp, wq_sb, QT_sb, 2)
            proj_slice(pp, wq_sb, QT_sb, 3)
            for g4 in range(4):
                make_v(pp, g4)

            # half-1 scores with den-h0 matmuls hidden in the tensor slack
            for kb in range(NB):
                scores_block(1, kb)
                if kb % 2 == 1:
                    den_mms(0, kb // 2)

        # ---- phase 2: fp8 DoubleRow ctx + den-h1 + epilogue ----
        with (
            tc.tile_pool(name="ctx_ps", bufs=2, space="PSUM") as cp,
            tc.tile_pool(name="d_ps", bufs=3, space="PSUM") as dp,
            tc.tile_pool(name="o_sb", bufs=4) as op,
        ):
            def ctx_half(h):
                return [
                    cp.tile([U, 512], F32, tag="ctx", name=f"ctx_ps_{h}_{j}")
                    for j in range(2)
                ]

            def ctx_mms(h, pair, ctx_ps):
                pr = h * NP + pair
                v2 = V_sb[:, 2 * pair:2 * pair + 2, :]
                for j in range(2):
                    nc.tensor.matmul(
                        ctx_ps[j][:],
                        lhsT=v2,
                        rhs=e_t[pr][:, :, j * 512:(j + 1) * 512],
                        start=(pair == 0),
                        stop=(pair == NP - 1),
                        perf_mode=DR,
                    )

            def den_finalize(h):
                # den rows -> SBUF, 8 PE transposes -> [128,8] -> reciprocal
                for j in range(2):
                    nc.vector.tensor_copy(
                        den_sb[h][:, j * 512:(j + 1) * 512], den_ps[(h, j)][0:1, :]
                    )
                rT = dp.tile([128, 512], F32, tag="d", name=f"rT_{h}")
                for i in range(8):
                    nc.tensor.matmul(
                        rT[:, i:i + 1],
                        lhsT=den_sb[h][:, i * 128:(i + 1) * 128],
                        rhs=ident_sb[:],
                        is_transpose=True,
                        skip_group_check=True,
                    )
                nc.vector.reciprocal(r_sb[:, h * 8:h * 8 + 8], rT[:, 0:8])

            def ctx_copy(h, j, ctx_ps):
                nc.vector.tensor_copy(
                    ctxT_sb[:, h * HQ + j * 512:h * HQ + (j + 1) * 512], ctx_ps[j][:]
                )

            def epilogue_qb(h, qb_local, store_eng, split_store=False,
                            via_scalar=False):
                qb = h * 8 + qb_local
                y_ps = dp.tile([128, D], F32, tag="d", name=f"y_{qb}")
                nc.tensor.matmul(
                    y_ps[:],
                    lhsT=ctxT_sb[:, qb * 128:(qb + 1) * 128],
                    rhs=wo_sb[:],
                    start=True,
                    stop=True,
                )
                o_t = op.tile([128, D], BF16, tag="o", name=f"o_{qb}")
                if via_scalar:
                    # ScalarE does y*r (per-partition scale), GpSimd adds the
                    # residual (SBUF-only) — keeps the final DVE chain short
                    nc.scalar.mul(o_t[:], y_ps[:], r_sb[:, qb:qb + 1])
                    nc.gpsimd.tensor_add(o_t[:], o_t[:], xb_sb[:, qb, :])
                else:
                    nc.vector.scalar_tensor_tensor(
                        o_t[:],
                        in0=y_ps[:],
                        scalar=r_sb[:, qb:qb + 1],
                        in1=xb_sb[:, qb, :],
                        op0=mybir.AluOpType.mult,
                        op1=mybir.AluOpType.add,
                    )
                dst = out_ext.ap()[qb * 128:(qb + 1) * 128, :]
                if split_store:
                    nc.sync.dma_start(dst[0:64], o_t[0:64])
                    nc.scalar.dma_start(dst[64:128], o_t[64:128])
                else:
                    store_eng.dma_start(dst, o_t[:])

            # ctx for half 0, straight through (exp long done); den-h0 was
            # fully accumulated during sH1, so finalize it under the ctx mms
            ctx0 = ctx_half(0)
            den_finalize(0)
            for pair in range(NP):
                ctx_mms(0, pair, ctx0)
            ctx_copy(0, 0, ctx0)
            ctx_copy(0, 1, ctx0)

            # den-h1 matmuls chase the exp chain 1:1 (they are the cheapest
            # consumers of each e-pair), with half-0 epilogues as tensor
            # filler; den finalization then overlaps the ctx-h1 stream
            ctx1 = ctx_half(1)
            for pr in range(NP):
                den_mms(1, pr)
                epilogue_qb(0, pr, nc.sync if pr % 2 == 0 else nc.scalar)
            den_finalize(1)
            for pr in range(NP):
                ctx_mms(1, pr, ctx1)
            ctx_copy(1, 0, ctx1)
            ctx_copy(1, 1, ctx1)
            for qb_local in range(8):
                epilogue_qb(
                    1, qb_local,
                    nc.sync if qb_local % 2 == 0 else nc.scalar,
                    split_store=(qb_local >= 6),
                    via_scalar=False,
                )


_NC_CACHE = {}


def _get_nc():
    if "nc" not in _NC_CACHE:
        _NC_CACHE["nc"] = build_attention_nc()
    return _NC_CACHE["nc"]


def prep_in_maps(inputs, W_q, W_k, W_v, W_o, b_o):
    """Host-side sharding + layout prep. One batch element per core."""
    B = inputs.shape[0]
    bf = ml_dtypes.bfloat16
    f8 = ml_dtypes.float8_e4m3

    def rearr_w8(w):  # [D, U] -> [128, (g t u)] fp8*W_SCALE, d = (2g+t)*128+p
        w8 = (np.asarray(w) * W_SCALE).astype(f8)
        return np.ascontiguousarray(
            w8.reshape(DC // 2, 2, 128, U).transpose(2, 0, 1, 3).reshape(128, D)
        )

    wq_r = rearr_w8(W_q)
    wk_r = rearr_w8(W_k)
    wv_r = rearr_w8(W_v)
    # fold the V-side residual scale (1/X_SCALE) into W_o
    wo_r = np.ascontiguousarray(np.asarray(W_o) / X_SCALE).astype(bf)
    bo = np.asarray(b_o, dtype=np.float32)

    in_maps = []
    for b in range(B):
        xf = np.asarray(inputs[b], dtype=np.float32)
        # xT fp8 host layout [p, ns, c, n5], scaled by X_SCALE
        xT4 = (
            (xf.T * X_SCALE).astype(f8)
            .reshape(DC, 128, NS, 512)
            .transpose(1, 2, 0, 3)
            .reshape(128, NS * DC * 512)
        )
        in_maps.append({
            "xb": np.ascontiguousarray(xf + bo).astype(bf),
            "xT": np.ascontiguousarray(xT4),
            "wq": wq_r,
            "wk": wk_r,
            "wv": wv_r,
            "wo": wo_r,
        })
    return in_maps


def kernel(inputs, W_q, W_k, W_v, W_o, b_o):
    in_maps = prep_in_maps(inputs, W_q, W_k, W_v, W_o, b_o)
    nc = _get_nc()
    res = run_bass_kernel_spmd(nc, in_maps, core_ids=list(range(len(in_maps))))
    return np.stack(
        [res.results[i]["out"].astype(np.float32) for i in range(len(in_maps))],
        axis=0,
    )
